# revision 17
# baseline (speedup 1.0000x reference)
"""Causal self-attention (token-shift + QK-RMSNorm + RoPE + value-residual)
Trainium2 Bass kernel, sharded over 8 NeuronCores.

Sharding: core c handles batch b=c//4 and head-group g=c%4 (4 heads, 512
channels). Each core computes q/k/v projections for its channels, attention
for its heads, and a partial c_proj (its 512 input rows of Wproj). Host sums
the 4 partials per batch and adds the residual.

v2 schedule: chunk j's projections are emitted interleaved with chunk j-1's
attention + c_proj so the tensor engine never stalls on the (scalar-engine)
softmax exponentials. Softmax denominators accumulate on the otherwise-idle
GpSimd engine (one ones-matmul per head/chunk instead of one per k-tile),
1/sum runs on the vector engine, and inputs are loaded with a handful of
big pre-tiled DMAs (xb chunk first so matmuls start immediately).
"""
import sys

sys.path.insert(0, "/opt/trn_rl_repo")

import numpy as np
import ml_dtypes

B, T, C, H, D = 2, 2048, 2048, 16, 128
NCORES = 8
LC = 512          # local channels per core (4 heads)
TQ = 512          # tq chunk size
NKT = C // 128    # 16 k-tiles over the C contraction
NCHUNK = T // TQ  # 4
ROPE_THETA = 10000.0
MASK_NEG = -1.0e5
EPS = float(np.finfo(np.float32).eps)

_bf = ml_dtypes.bfloat16

_prog_cache = {}


def _build_program():
    import concourse.bass as bass
    import concourse.mybir as mybir
    from concourse import bacc
    from concourse.tile import TileContext
    from concourse.alu_op_type import AluOpType

    AFt = mybir.ActivationFunctionType
    if not getattr(bacc, "_act_tables_pinned", False):
        _orig_gat = bacc.get_activation_tables

        def _pinned_gat(arch):
            tables = _orig_gat(arch)
            pinned = {AFt.Ln, AFt.Exp, AFt.Square, AFt.Copy, AFt.Identity}
            for name, fns in tables.items():
                if name != "natural_log_exp_and_others":
                    fns -= pinned
            return tables

        bacc.get_activation_tables = _pinned_gat
        bacc._act_tables_pinned = True

    F32 = mybir.dt.float32
    BF16 = mybir.dt.bfloat16
    AF = mybir.ActivationFunctionType

    nc = bacc.Bacc("TRN2", target_bir_lowering=False, debug=False)

    # pre-tiled DRAM inputs (partition-major; wq additionally m-major so the
    # first projection can start after a 0.5MB transfer)
    xbig = nc.dram_tensor("xbig", [128, NCHUNK, NKT, TQ], BF16,
                          kind="ExternalInput").ap()
    wq = nc.dram_tensor("wq", [4, 128, NKT, 128], BF16,
                        kind="ExternalInput").ap()
    wk = nc.dram_tensor("wk", [128, NKT, LC], BF16, kind="ExternalInput").ap()
    wv = nc.dram_tensor("wv", [128, NKT, LC], BF16, kind="ExternalInput").ap()
    wp = nc.dram_tensor("wp", [128, 4, C], BF16, kind="ExternalInput").ap()
    v1big = nc.dram_tensor("v1big", [128, NCHUNK, 4, LC], BF16,
                           kind="ExternalInput").ap()
    cos2 = nc.dram_tensor("cos2", [128, T], BF16, kind="ExternalInput").ap()
    sin2 = nc.dram_tensor("sin2", [128, T], BF16, kind="ExternalInput").ap()
    masks = nc.dram_tensor("masks", [128, 896], F32, kind="ExternalInput").ap()
    perm = nc.dram_tensor("perm", [128, 128], BF16, kind="ExternalInput").ap()
    F16 = mybir.dt.float16
    outT = nc.dram_tensor("outT", [C, T], F16, kind="ExternalOutput").ap()

    SCALE = 1.0 / float(np.sqrt(D))

    with TileContext(nc) as tc:
        with (
            tc.tile_pool(name="cpool", bufs=1) as cpool,
            tc.tile_pool(name="kvpool", bufs=1) as kvpool,
            tc.tile_pool(name="xpool", bufs=2) as xpool,
            tc.tile_pool(name="qpool", bufs=2) as qpool,
            tc.tile_pool(name="apool", bufs=1) as apool,
            tc.tile_pool(name="epool", bufs=3) as epool,
            tc.tile_pool(name="wpool", bufs=2) as wpool,
            tc.tile_pool(name="opool", bufs=3) as opool,
            tc.tile_pool(name="pspool", bufs=1, space="PSUM") as pspool,
        ):
            # ---------------- prologue DMAs (multi-engine issue) ----------
            xb_sb = {}
            v1_sb = {}

            # tiny constants on the gpsimd (SWDGE) queue so they don't delay
            # the weight/activation streams on the SP/Act queues
            perm_sb = cpool.tile([128, 128], BF16, tag="perm", name="perm_sb")
            nc.gpsimd.dma_start(perm_sb, perm)
            mask_sb = cpool.tile([128, 896], F32, tag="mask", name="mask_sb")
            nc.gpsimd.dma_start(mask_sb, masks)
            cos_sb = cpool.tile([128, T], BF16, tag="cos", name="cos_sb")
            nc.gpsimd.dma_start(cos_sb, cos2)
            sin_sb = cpool.tile([128, T], BF16, tag="sin", name="sin_sb")
            nc.gpsimd.dma_start(sin_sb, sin2)

            # xb chunk 0 split in 4 so the first quad starts ~2.5us in
            xb_sb[0] = xpool.tile([128, NKT, TQ], BF16, tag="xb", bufs=2,
                                  name="xb0")
            for kq in range(4):
                nc.sync.dma_start(xb_sb[0][:, 4 * kq:4 * kq + 4, :],
                                  xbig[:, 0, 4 * kq:4 * kq + 4, :])

            wq_sb = cpool.tile([128, 4, NKT, 128], BF16, tag="wq",
                               name="wq_sb")
            for m in range(4):
                nc.scalar.dma_start(wq_sb[:, m], wq[m])

            wk_sb = cpool.tile([128, NKT, LC], BF16, tag="wk", name="wk_sb")
            nc.sync.dma_start(wk_sb, wk)
            wv_sb = cpool.tile([128, NKT, LC], BF16, tag="wv", name="wv_sb")
            nc.scalar.dma_start(wv_sb, wv)
            wp_sb = cpool.tile([128, 4, C], BF16, tag="wp", name="wp_sb")
            nc.scalar.dma_start(wp_sb, wp)

            v1_sb[0] = wpool.tile([128, 4, LC], BF16, tag="v1", bufs=2,
                                  name="v1_0")
            nc.sync.dma_start(v1_sb[0], v1big[:, 0])

            ones = cpool.tile([128, 128], BF16, tag="ones", name="ones")
            nc.vector.memset(ones, 1.0)
            epst = cpool.tile([128, 1], F32, tag="epst", name="epst")
            nc.vector.memset(epst, EPS)

            # persistent per-(head, chunk) K^T and per-chunk V tiles
            kTc = {}   # (h, jc) -> [128, TQ] tile
            vst = {}   # tkc -> [128, LC] tile
            qT_saved = {}   # (j, h) -> tile
            at_saved = {}   # (j, h) -> tile
            eacc = {}  # h -> tile (rotated per block via tag reuse)

            # PSUM score-bank rotation helper. Blocks 1-3: tag "s" (2 bufs).
            # Block 4 (no proj filler, lag 2): rotate s/ss/s/swp for depth 4.
            s_rot_state = {"i": 0, "rot": ("s",)}

            def claim_s(nm):
                rot = s_rot_state["rot"]
                tag = rot[s_rot_state["i"] % len(rot)]
                s_rot_state["i"] += 1
                return pspool.tile([128, TQ], F32, tag=tag,
                                   bufs=2 if tag == "s" else 1, name=nm)

            # ---------------- emission unit builders ----------------------
            def prefetch_unit(j):
                def u():
                    xb_sb[j] = xpool.tile([128, NKT, TQ], BF16, tag="xb",
                                          bufs=2, name=f"xb{j}")
                    nc.sync.dma_start(xb_sb[j], xbig[:, j])
                    v1_sb[j] = wpool.tile([128, 4, LC], BF16, tag="v1",
                                          bufs=2, name=f"v1_{j}")
                    nc.sync.dma_start(v1_sb[j], v1big[:, j])
                return [u]

            def qk_group_units(j, which, m):
                tq0 = TQ * j
                st = {}
                units = []

                def quad(qi):
                    def u():
                        if qi == 0:
                            st["ps"] = pspool.tile(
                                [128, TQ], F32, tag="mm", bufs=2,
                                name=f"{which}ps{m}_{j}")
                        for kt in range(4 * qi, 4 * qi + 4):
                            lhsT = (wq_sb[:, m, kt, :] if which == "q"
                                    else wk_sb[:, kt, 128 * m:128 * m + 128])
                            nc.tensor.matmul(
                                st["ps"],
                                lhsT,
                                xb_sb[j][:, kt, :],
                                start=(kt == 0),
                                stop=(kt == NKT - 1),
                            )
                    return u

                units += [quad(qi) for qi in range(4)]

                def chain():
                    q_ps = st["ps"]
                    q_sb = wpool.tile([128, TQ], BF16, tag="qsb",
                                      name=f"{which}sb{m}_{j}")
                    sq = wpool.tile([128, TQ], BF16, tag="sq",
                                    name=f"{which}sq{m}_{j}")
                    if which == "q":
                        nc.vector.tensor_copy(q_sb, q_ps)
                        nc.scalar.square(sq, q_sb)
                    else:
                        nc.scalar.copy(q_sb, q_ps)
                        nc.vector.tensor_mul(sq, q_sb, q_sb)
                    ss_ps = pspool.tile([128, TQ], F32, tag="ss", bufs=1,
                                        name=f"{which}ss{m}_{j}")
                    nc.tensor.matmul(ss_ps, ones, sq, start=True, stop=True)
                    lnt = wpool.tile([128, TQ], F32, tag="lnt",
                                     name=f"{which}ln{m}_{j}")
                    nc.scalar.activation(lnt, ss_ps, AF.Ln,
                                         scale=1.0 / D, bias=epst)
                    rms = wpool.tile([128, TQ], BF16, tag="rms",
                                     name=f"{which}rms{m}_{j}")
                    nc.scalar.activation(rms, lnt, AF.Exp, scale=-0.5)
                    sw_ps = pspool.tile([128, TQ], F32, tag="swp", bufs=1,
                                        name=f"{which}swp{m}_{j}")
                    nc.tensor.matmul(sw_ps, perm_sb, q_sb,
                                     start=True, stop=True)
                    t1 = wpool.tile([128, TQ], BF16, tag="t1",
                                    name=f"{which}t1{m}_{j}")
                    nc.vector.tensor_mul(t1, q_sb, cos_sb[:, tq0:tq0 + TQ])
                    t2 = wpool.tile([128, TQ], BF16, tag="t2",
                                    name=f"{which}t2{m}_{j}")
                    nc.vector.tensor_mul(t2, sw_ps, sin_sb[:, tq0:tq0 + TQ])
                    t3 = wpool.tile([128, TQ], BF16, tag="t3",
                                    name=f"{which}t3{m}_{j}")
                    nc.gpsimd.tensor_add(t3, t1, t2)
                    if which == "q":
                        dst = qpool.tile([128, TQ], BF16, tag=f"qT{m}",
                                         bufs=2, name=f"qT{m}_{j}")
                        qT_saved[(j, m)] = dst
                    else:
                        dst = kvpool.tile([128, TQ], BF16, tag=f"kT{m}_{j}",
                                          bufs=1, name=f"kT{m}_{j}")
                        kTc[(m, j)] = dst
                    nc.gpsimd.tensor_mul(dst, t3, rms)

                units.append(chain)
                return units

            def v_group_units(j, tt):
                st = {}
                units = []

                def quad(qi):
                    def u():
                        if qi == 0:
                            st["ps"] = pspool.tile(
                                [128, LC], F32, tag="mm", bufs=2,
                                name=f"vps{tt}_{j}")
                        for kt in range(4 * qi, 4 * qi + 4):
                            nc.tensor.matmul(
                                st["ps"],
                                xb_sb[j][:, kt, 128 * tt:128 * tt + 128],
                                wv_sb[:, kt, :],
                                start=(kt == 0),
                                stop=(kt == NKT - 1),
                            )
                    return u

                units += [quad(qi) for qi in range(4)]

                def blend():
                    vt = kvpool.tile([128, LC], BF16, tag=f"v{4 * j + tt}",
                                     bufs=1, name=f"v{4 * j + tt}")
                    nc.vector.tensor_add(vt, st["ps"], v1_sb[j][:, tt, :])
                    vst[4 * j + tt] = vt

                units.append(blend)
                return units

            def attn_pair_units(jm1, pair, lag):
                ntk = 4 * (jm1 + 1)
                tq0 = TQ * jm1
                st = {}
                units = []

                def tile_c0(tk):
                    # diagonal tiles: columns q < 128*tk - tq0 are fully
                    # masked; skip them in scores/exp/accumulate/pv
                    return max(0, 128 * tk - tq0)

                def consume(h, tkl):
                    s_t = st[("s", h, tkl)]
                    c0 = tile_c0(tkl)
                    if tkl >= 4 * jm1:  # diagonal tile: causal mask add
                        nc.vector.tensor_add(
                            s_t[:, c0:], s_t[:, c0:],
                            mask_sb[:, 384:384 + TQ - c0])
                    e_t = epool.tile([128, TQ], BF16, tag=f"e{h % 2}",
                                     bufs=3, name=f"e{h}_{tkl}_{jm1}")
                    nc.scalar.activation(e_t[:, c0:], s_t[:, c0:],
                                         AF.Exp, scale=SCALE)
                    st[("e", h, tkl)] = e_t
                    if tkl == 0:
                        eacc[h] = wpool.tile([128, TQ], F32, tag=f"eacc{h}",
                                             bufs=1, name=f"eacc{h}_{jm1}")
                        nc.vector.tensor_copy(eacc[h], e_t)
                    elif (h + tkl) % 2 == 0:
                        nc.gpsimd.tensor_add(eacc[h][:, c0:], eacc[h][:, c0:],
                                             e_t[:, c0:])
                    else:
                        nc.vector.tensor_add(eacc[h][:, c0:], eacc[h][:, c0:],
                                             e_t[:, c0:])

                def mk_round(tk):
                    def u():
                        tkl = tk - lag
                        if tkl >= 0:
                            for h in pair:
                                consume(h, tkl)
                        if tk < ntk:
                            c0 = tile_c0(tk)
                            for h in pair:
                                s_t = claim_s(f"s{h}_{tk}_{jm1}")
                                nc.tensor.matmul(
                                    s_t[:, c0:],
                                    kTc[(h, tk // 4)][
                                        :, 128 * (tk % 4):128 * (tk % 4) + 128],
                                    qT_saved[(jm1, h)][:, c0:],
                                    start=True,
                                    stop=True,
                                )
                                st[("s", h, tk)] = s_t
                        if tkl >= 0:
                            c0 = tile_c0(tkl)
                            for h in pair:
                                if tkl == 0:
                                    st[("pv", h)] = pspool.tile(
                                        [128, TQ], F32, tag="pv", bufs=2,
                                        name=f"pv{h}_{jm1}")
                                nc.tensor.matmul(
                                    st[("pv", h)][:, c0:],
                                    vst[tkl][:, 128 * h:128 * h + 128],
                                    st[("e", h, tkl)][:, c0:],
                                    start=(tkl == 0),
                                    stop=(tkl == ntk - 1),
                                    skip_group_check=(c0 > 0),
                                )
                    return u

                units += [mk_round(tk) for tk in range(ntk + lag)]

                def tail():
                    for h in pair:
                        accb = wpool.tile([128, TQ], BF16, tag="accb",
                                          name=f"accb{h}_{jm1}")
                        nc.scalar.copy(accb, eacc[h])
                        se_ps = claim_s(f"se{h}_{jm1}")
                        nc.tensor.matmul(se_ps, ones, accb,
                                         start=True, stop=True)
                        lnse = wpool.tile([128, TQ], F32, tag="lnse",
                                          name=f"lnse{h}_{jm1}")
                        nc.scalar.activation(lnse, se_ps, AF.Ln)
                        rec = wpool.tile([128, TQ], BF16, tag="rec",
                                         name=f"rec{h}_{jm1}")
                        nc.scalar.activation(rec, lnse, AF.Exp, scale=-1.0)
                        at = apool.tile([128, TQ], BF16, tag=f"at{h}",
                                        bufs=2, name=f"at{h}_{jm1}")
                        nc.vector.tensor_mul(at, st[("pv", h)], rec)
                        at_saved[(jm1, h)] = at

                units.append(tail)
                return units

            def cproj_units(jm1):
                tq0 = TQ * jm1
                units = []
                for co in range(16):
                    def u(co=co):
                        o_ps = pspool.tile([128, TQ], F32, tag="mm", bufs=2,
                                           name=f"ops{co}_{jm1}")
                        for m in range(4):
                            nc.tensor.matmul(
                                o_ps,
                                wp_sb[:, m, 128 * co:128 * co + 128],
                                at_saved[(jm1, m)],
                                start=(m == 0),
                                stop=(m == 3),
                            )
                        o_sb = opool.tile([128, TQ], F16, tag="osb",
                                          name=f"osb{co}_{jm1}")
                        if co % 2 == 0:
                            nc.vector.tensor_copy(o_sb, o_ps)
                        else:
                            nc.scalar.copy(o_sb, o_ps)
                        nc.sync.dma_start(
                            outT[128 * co:128 * co + 128, tq0:tq0 + TQ], o_sb)
                    units.append(u)
                return units

            def run_units(a_units, b_units):
                # units are (closure, weight); merge by cumulative weight
                if not a_units or not b_units:
                    for u, _ in a_units + b_units:
                        u()
                    return
                wa = sum(w for _, w in a_units)
                wb = sum(w for _, w in b_units)
                seq = []
                acc = 0.0
                for i, (u, w) in enumerate(a_units):
                    acc += w
                    seq.append((acc / wa, 0, i, u))
                acc = 0.0
                for i, (u, w) in enumerate(b_units):
                    acc += w
                    seq.append(((acc - 0.5 * w) / wb, 1, i, u))
                seq.sort(key=lambda t: (t[0], t[1], t[2]))
                for _, _, _, u in seq:
                    u()

            def weighted(units, w):
                return [(u, w) for u in units]

            # ---------------- blocks ---------------------------------------
            for j in range(NCHUNK + 1):
                b_units = []
                if j < NCHUNK:
                    if j + 1 < NCHUNK:
                        b_units += weighted(prefetch_unit(j + 1), 0.3)
                    # software-pipeline the rms/rope chains one group back so
                    # the PE never head-of-line blocks on a chain's inputs
                    groups = [qk_group_units(j, "q", m) for m in range(4)]
                    groups += [qk_group_units(j, "k", m) for m in range(4)]
                    groups += [v_group_units(j, tt) for tt in range(4)]
                    pending = None
                    for g in groups:
                        b_units += weighted(g[:-1], 1.0)
                        if pending is not None:
                            b_units += weighted([pending], 1.6)
                        pending = g[-1]
                    b_units += weighted([pending], 1.6)
                else:
                    # last block: chunk-2 c_proj provides the PE filler
                    b_units += weighted(cproj_units(2), 1.0)
                a_units = []
                if j >= 1:
                    lag = 1 if j < NCHUNK else 2
                    if j == NCHUNK:
                        s_rot_state["rot"] = ("s", "ss", "s", "swp")
                    for pair in ((0, 1), (2, 3)):
                        pu = attn_pair_units(j - 1, pair, lag)
                        a_units += weighted(pu[:-1], 1.5)
                        a_units += weighted([pu[-1]], 2.5)
                    if j in (1, 2):
                        a_units += weighted(cproj_units(j - 1), 1.0)
                    elif j == NCHUNK:
                        a_units += weighted(cproj_units(3), 1.0)
                run_units(a_units, b_units)

    nc.finalize()
    return nc


def _host_prep(inputs):
    """Build the 8 per-core input maps (all numpy)."""
    x = np.asarray(inputs["x"], np.float32)
    v1 = np.asarray(inputs["v1"], np.float32)
    x_q = np.asarray(inputs["x_q"], np.float32)
    x_k = np.asarray(inputs["x_k"], np.float32)
    x_v = np.asarray(inputs["x_v"], np.float32)
    Wq = np.asarray(inputs["Wq"], np.float32)
    Wk = np.asarray(inputs["Wk"], np.float32)
    Wv = np.asarray(inputs["Wv"], np.float32)
    Wproj = np.asarray(inputs["Wproj"], np.float32)
    lamb = float(np.asarray(inputs["lamb"]))

    assert np.array_equal(x_q, x_k) and np.array_equal(x_q, x_v), (
        "kernel assumes shared token-shift mix vectors (x_q == x_k == x_v)"
    )

    # token-shift blend, then transpose per batch; pre-tile partition-major:
    # xbig[p, j, kt, t] = xb[b].T[128*kt + p, 512*j + t]
    sh = np.concatenate([np.zeros((B, 1, C), np.float32), x[:, :-1]], axis=1)
    xb = x * (1.0 - x_q) + sh * x_q
    xbig = []
    for b_ in range(B):
        xt = xb[b_].T.astype(_bf)                      # [C, T]
        xt = xt.reshape(NKT, 128, NCHUNK, TQ)          # [kt, p, j, t]
        xbig.append(np.ascontiguousarray(xt.transpose(1, 2, 0, 3)))

    def tile_w(w):  # [C, LC] -> [128, NKT, LC]
        return np.ascontiguousarray(
            w.reshape(NKT, 128, LC).transpose(1, 0, 2))

    # rope tables, duplicated halves; sin second half negated
    inv = 1.0 / (ROPE_THETA ** (np.arange(0, D, 2, dtype=np.float32) / D))
    fr = np.outer(np.arange(T, dtype=np.float32), inv)  # [T, 64]
    cosT = np.cos(fr).T.astype(np.float32)  # [64, T]
    sinT = np.sin(fr).T.astype(np.float32)
    cos2 = np.concatenate([cosT, cosT], axis=0).astype(_bf)
    sin2 = np.concatenate([sinT, -sinT], axis=0).astype(_bf)

    # causal mask master strip: M[p, g] = 0 if g >= p + 384 else MASK_NEG
    p = np.arange(128)[:, None]
    g = np.arange(896)[None, :]
    masks = np.where(g >= p + 384, 0.0, MASK_NEG).astype(np.float32)
    permm = np.roll(np.eye(128, dtype=np.float32), 64, axis=0).astype(_bf)

    in_maps = []
    for c in range(NCORES):
        b_ = c // 4
        g_ = c % 4
        L = slice(LC * g_, LC * g_ + LC)
        # v1big[p, j, tt, c] = (lamb*v1)[512*j + 128*tt + p, L]
        v1l = (lamb * v1[b_][:, L]).astype(_bf)        # [T, LC]
        v1b = np.ascontiguousarray(
            v1l.reshape(NCHUNK, 4, 128, LC).transpose(2, 0, 1, 3))
        # wp[p, m, c] = Wproj[:, L].T[128*m + p, :]
        wpl = np.ascontiguousarray(Wproj[:, L].T)      # [LC, C]
        wpb = np.ascontiguousarray(
            wpl.reshape(4, 128, C).transpose(1, 0, 2)).astype(_bf)
        wq_t = tile_w(np.ascontiguousarray(Wq[L].T).astype(_bf))
        wq_m = np.ascontiguousarray(
            wq_t.reshape(128, NKT, 4, 128).transpose(2, 0, 1, 3))
        in_maps.append({
            "xbig": xbig[b_],
            "wq": wq_m,
            "wk": tile_w(np.ascontiguousarray(Wk[L].T).astype(_bf)),
            "wv": tile_w(
                np.ascontiguousarray(((1.0 - lamb) * Wv[L]).T).astype(_bf)),
            "wp": wpb,
            "v1big": v1b,
            "cos2": cos2,
            "sin2": sin2,
            "masks": masks,
            "perm": permm,
        })
    return in_maps


def _run(in_maps, trace=False):
    from concourse.bass_utils import run_bass_kernel_spmd

    if "nc" not in _prog_cache:
        _prog_cache["nc"] = _build_program()
    return run_bass_kernel_spmd(
        _prog_cache["nc"], in_maps, core_ids=list(range(NCORES)), trace=trace
    )


def kernel(**inputs) -> np.ndarray:
    residual = np.asarray(inputs["residual"], np.float32)
    in_maps = _host_prep(inputs)
    res = _run(in_maps)
    out = np.empty((B, T, C), np.float32)
    for b_ in range(B):
        acc = res.results[4 * b_]["outT"].astype(np.float32)
        for g_ in range(1, 4):
            acc = acc + res.results[4 * b_ + g_]["outT"]
        out[b_] = residual[b_] + acc.T
    return out


# revision 27
# speedup vs baseline: 1.0368x; 1.0368x over previous
"""Causal self-attention (token-shift + QK-RMSNorm + RoPE + value-residual)
Trainium2 Bass kernel, sharded over 8 NeuronCores.

Sharding: core c handles batch b=c//4 and head-group g=c%4 (4 heads, 512
channels). Each core computes q/k/v projections for its channels, attention
for its heads, and a partial c_proj (its 512 input rows of Wproj). Host sums
the 4 partials per batch and adds the residual.

v2 schedule: chunk j's projections are emitted interleaved with chunk j-1's
attention + c_proj so the tensor engine never stalls on the (scalar-engine)
softmax exponentials. Softmax denominators accumulate on the otherwise-idle
GpSimd engine (one ones-matmul per head/chunk instead of one per k-tile),
1/sum runs on the vector engine, and inputs are loaded with a handful of
big pre-tiled DMAs (xb chunk first so matmuls start immediately).
"""
import sys

sys.path.insert(0, "/opt/trn_rl_repo")

import numpy as np
import ml_dtypes

B, T, C, H, D = 2, 2048, 2048, 16, 128
NCORES = 8
LC = 512          # local channels per core (4 heads)
TQ = 512          # tq chunk size
NKT = C // 128    # 16 k-tiles over the C contraction
NCHUNK = T // TQ  # 4
ROPE_THETA = 10000.0
MASK_NEG = -1.0e5
EPS = float(np.finfo(np.float32).eps)

_bf = ml_dtypes.bfloat16

_prog_cache = {}


def _build_program():
    import concourse.bass as bass
    import concourse.mybir as mybir
    from concourse import bacc
    from concourse.tile import TileContext
    from concourse.alu_op_type import AluOpType

    AFt = mybir.ActivationFunctionType
    if not getattr(bacc, "_act_tables_pinned", False):
        _orig_gat = bacc.get_activation_tables

        def _pinned_gat(arch):
            tables = _orig_gat(arch)
            pinned = {AFt.Ln, AFt.Exp, AFt.Square, AFt.Copy, AFt.Identity}
            for name, fns in tables.items():
                if name != "natural_log_exp_and_others":
                    fns -= pinned
            return tables

        bacc.get_activation_tables = _pinned_gat
        bacc._act_tables_pinned = True

    F32 = mybir.dt.float32
    F32R = mybir.dt.float32r
    BF16 = mybir.dt.bfloat16
    AF = mybir.ActivationFunctionType

    nc = bacc.Bacc("TRN2", target_bir_lowering=False, debug=False)

    # pre-tiled DRAM inputs (partition-major; wq additionally m-major so the
    # first projection can start after a 0.5MB transfer)
    xbig = nc.dram_tensor("xbig", [128, NCHUNK, NKT, TQ], BF16,
                          kind="ExternalInput").ap()
    wq = nc.dram_tensor("wq", [4, 128, NKT, 128], BF16,
                        kind="ExternalInput").ap()
    wk = nc.dram_tensor("wk", [128, NKT, LC], BF16, kind="ExternalInput").ap()
    wv = nc.dram_tensor("wv", [128, NKT, LC], BF16, kind="ExternalInput").ap()
    wp = nc.dram_tensor("wp", [128, 4, C], BF16, kind="ExternalInput").ap()
    v1big = nc.dram_tensor("v1big", [128, NCHUNK, 4, LC], BF16,
                           kind="ExternalInput").ap()
    cos2 = nc.dram_tensor("cos2", [128, T], BF16, kind="ExternalInput").ap()
    sin2 = nc.dram_tensor("sin2", [128, T], BF16, kind="ExternalInput").ap()
    masks = nc.dram_tensor("masks", [128, 896], F32, kind="ExternalInput").ap()
    perm = nc.dram_tensor("perm", [128, 128], BF16, kind="ExternalInput").ap()
    ones32d = nc.dram_tensor("ones32", [128, 128], F32R,
                             kind="ExternalInput").ap()
    F16 = mybir.dt.float16
    outT = nc.dram_tensor("outT", [C, T], F16, kind="ExternalOutput").ap()

    SCALE = 1.0 / float(np.sqrt(D))

    with TileContext(nc) as tc:
        with (
            tc.tile_pool(name="cpool", bufs=1) as cpool,
            tc.tile_pool(name="kvpool", bufs=1) as kvpool,
            tc.tile_pool(name="xpool", bufs=2) as xpool,
            tc.tile_pool(name="qpool", bufs=2) as qpool,
            tc.tile_pool(name="apool", bufs=1) as apool,
            tc.tile_pool(name="epool", bufs=3) as epool,
            tc.tile_pool(name="wpool", bufs=2) as wpool,
            tc.tile_pool(name="opool", bufs=3) as opool,
            tc.tile_pool(name="pspool", bufs=1, space="PSUM") as pspool,
        ):
            # ---------------- prologue DMAs (multi-engine issue) ----------
            xb_sb = {}
            v1_sb = {}

            # tiny constants on the gpsimd (SWDGE) queue so they don't delay
            # the weight/activation streams on the SP/Act queues
            perm_sb = cpool.tile([128, 128], BF16, tag="perm", name="perm_sb")
            nc.gpsimd.dma_start(perm_sb, perm)
            mask_sb = cpool.tile([128, 896], F32, tag="mask", name="mask_sb")
            nc.gpsimd.dma_start(mask_sb, masks)
            cos_sb = cpool.tile([128, T], BF16, tag="cos", name="cos_sb")
            nc.gpsimd.dma_start(cos_sb, cos2)
            sin_sb = cpool.tile([128, T], BF16, tag="sin", name="sin_sb")
            nc.gpsimd.dma_start(sin_sb, sin2)

            # xb chunk 0 split in 4 so the first quad starts ~2.5us in
            xb_sb[0] = xpool.tile([128, NKT, TQ], BF16, tag="xb", bufs=2,
                                  name="xb0")
            for kq in range(4):
                nc.sync.dma_start(xb_sb[0][:, 4 * kq:4 * kq + 4, :],
                                  xbig[:, 0, 4 * kq:4 * kq + 4, :])

            wq_sb = cpool.tile([128, 4, NKT, 128], BF16, tag="wq",
                               name="wq_sb")
            for m in range(4):
                nc.scalar.dma_start(wq_sb[:, m], wq[m])

            wk_sb = cpool.tile([128, NKT, LC], BF16, tag="wk", name="wk_sb")
            nc.sync.dma_start(wk_sb, wk)
            wv_sb = cpool.tile([128, NKT, LC], BF16, tag="wv", name="wv_sb")
            nc.scalar.dma_start(wv_sb, wv)
            wp_sb = cpool.tile([128, 4, C], BF16, tag="wp", name="wp_sb")
            nc.scalar.dma_start(wp_sb, wp)

            v1_sb[0] = wpool.tile([128, 4, LC], BF16, tag="v1", bufs=2,
                                  name="v1_0")
            nc.sync.dma_start(v1_sb[0], v1big[:, 0])

            ones = cpool.tile([128, 128], BF16, tag="ones", name="ones")
            nc.vector.memset(ones, 1.0)
            ones32 = cpool.tile([128, 128], F32R, tag="ones32", name="ones32")
            nc.gpsimd.dma_start(ones32, ones32d)
            epst = cpool.tile([128, 1], F32, tag="epst", name="epst")
            nc.vector.memset(epst, EPS)

            # persistent per-(head, chunk) K^T and per-chunk V tiles
            kTc = {}   # (h, jc) -> [128, TQ] tile
            vst = {}   # tkc -> [128, LC] tile
            qT_saved = {}   # (j, h) -> tile
            at_saved = {}   # (j, h) -> tile
            eacc = {}  # h -> tile (rotated per block via tag reuse)

            # PSUM score-bank rotation helper. Blocks 1-3: tag "s" (2 bufs).
            # Block 4 (no proj filler, lag 2): rotate s/ss/s/swp for depth 4.
            s_rot_state = {"i": 0, "rot": ("s",)}

            def claim_s(nm):
                rot = s_rot_state["rot"]
                tag = rot[s_rot_state["i"] % len(rot)]
                s_rot_state["i"] += 1
                return pspool.tile([128, TQ], F32, tag=tag,
                                   bufs=2 if tag == "s" else 1, name=nm)

            # ---------------- emission unit builders ----------------------
            def prefetch_unit(j):
                def u():
                    xb_sb[j] = xpool.tile([128, NKT, TQ], BF16, tag="xb",
                                          bufs=2, name=f"xb{j}")
                    nc.sync.dma_start(xb_sb[j], xbig[:, j])
                    v1_sb[j] = wpool.tile([128, 4, LC], BF16, tag="v1",
                                          bufs=2, name=f"v1_{j}")
                    nc.sync.dma_start(v1_sb[j], v1big[:, j])
                return [u]

            def qk_group_units(j, which, m):
                tq0 = TQ * j
                st = {}
                units = []

                def quad(qi):
                    def u():
                        if qi == 0:
                            st["ps"] = pspool.tile(
                                [128, TQ], F32, tag="mm", bufs=2,
                                name=f"{which}ps{m}_{j}")
                        for kt in range(4 * qi, 4 * qi + 4):
                            lhsT = (wq_sb[:, m, kt, :] if which == "q"
                                    else wk_sb[:, kt, 128 * m:128 * m + 128])
                            nc.tensor.matmul(
                                st["ps"],
                                lhsT,
                                xb_sb[j][:, kt, :],
                                start=(kt == 0),
                                stop=(kt == NKT - 1),
                            )
                    return u

                units += [quad(qi) for qi in range(4)]

                def chain():
                    q_ps = st["ps"]
                    q_sb = wpool.tile([128, TQ], BF16, tag="qsb",
                                      name=f"{which}sb{m}_{j}")
                    sq = wpool.tile([128, TQ], BF16, tag="sq",
                                    name=f"{which}sq{m}_{j}")
                    if which == "q":
                        nc.vector.tensor_copy(q_sb, q_ps)
                        nc.scalar.square(sq, q_sb)
                    else:
                        nc.scalar.copy(q_sb, q_ps)
                        nc.vector.tensor_mul(sq, q_sb, q_sb)
                    ss_ps = pspool.tile([128, TQ], F32, tag="ss", bufs=1,
                                        name=f"{which}ss{m}_{j}")
                    nc.tensor.matmul(ss_ps, ones, sq, start=True, stop=True)
                    lnt = wpool.tile([128, TQ], F32, tag="lnt",
                                     name=f"{which}ln{m}_{j}")
                    nc.scalar.activation(lnt, ss_ps, AF.Ln,
                                         scale=1.0 / D, bias=epst)
                    rms = wpool.tile([128, TQ], BF16, tag="rms",
                                     name=f"{which}rms{m}_{j}")
                    nc.scalar.activation(rms, lnt, AF.Exp, scale=-0.5)
                    sw_ps = pspool.tile([128, TQ], F32, tag="swp", bufs=1,
                                        name=f"{which}swp{m}_{j}")
                    nc.tensor.matmul(sw_ps, perm_sb, q_sb,
                                     start=True, stop=True)
                    t1 = wpool.tile([128, TQ], BF16, tag="t1",
                                    name=f"{which}t1{m}_{j}")
                    nc.vector.tensor_mul(t1, q_sb, cos_sb[:, tq0:tq0 + TQ])
                    t2 = wpool.tile([128, TQ], BF16, tag="t2",
                                    name=f"{which}t2{m}_{j}")
                    nc.vector.tensor_mul(t2, sw_ps, sin_sb[:, tq0:tq0 + TQ])
                    t3 = wpool.tile([128, TQ], BF16, tag="t3",
                                    name=f"{which}t3{m}_{j}")
                    nc.gpsimd.tensor_add(t3, t1, t2)
                    if which == "q":
                        dst = qpool.tile([128, TQ], BF16, tag=f"qT{m}",
                                         bufs=2, name=f"qT{m}_{j}")
                        qT_saved[(j, m)] = dst
                    else:
                        dst = kvpool.tile([128, TQ], BF16, tag=f"kT{m}_{j}",
                                          bufs=1, name=f"kT{m}_{j}")
                        kTc[(m, j)] = dst
                    nc.gpsimd.tensor_mul(dst, t3, rms)

                units.append(chain)
                return units

            def v_group_units(j, tt):
                st = {}
                units = []

                def quad(qi):
                    def u():
                        if qi == 0:
                            st["ps"] = pspool.tile(
                                [128, LC], F32, tag="mm", bufs=2,
                                name=f"vps{tt}_{j}")
                        for kt in range(4 * qi, 4 * qi + 4):
                            nc.tensor.matmul(
                                st["ps"],
                                xb_sb[j][:, kt, 128 * tt:128 * tt + 128],
                                wv_sb[:, kt, :],
                                start=(kt == 0),
                                stop=(kt == NKT - 1),
                            )
                    return u

                units += [quad(qi) for qi in range(4)]

                def blend():
                    vt = kvpool.tile([128, LC], BF16, tag=f"v{4 * j + tt}",
                                     bufs=1, name=f"v{4 * j + tt}")
                    nc.vector.tensor_add(vt, st["ps"], v1_sb[j][:, tt, :])
                    vst[4 * j + tt] = vt

                units.append(blend)
                return units

            def attn_pair_units(jm1, pair, lag):
                ntk = 4 * (jm1 + 1)
                tq0 = TQ * jm1
                st = {}
                units = []

                def tile_c0(tk):
                    # diagonal tiles: columns q < 128*tk - tq0 are fully
                    # masked; skip them in scores/exp/accumulate/pv
                    return max(0, 128 * tk - tq0)

                def consume(h, tkl):
                    s_t = st[("s", h, tkl)]
                    c0 = tile_c0(tkl)
                    if tkl >= 4 * jm1:  # diagonal tile: causal mask add
                        nc.vector.tensor_add(
                            s_t[:, c0:], s_t[:, c0:],
                            mask_sb[:, 384:384 + TQ - c0])
                    e_t = epool.tile([128, TQ], BF16, tag=f"e{h % 2}",
                                     bufs=3, name=f"e{h}_{tkl}_{jm1}")
                    nc.scalar.activation(e_t[:, c0:], s_t[:, c0:],
                                         AF.Exp, scale=SCALE)
                    st[("e", h, tkl)] = e_t
                    # blocks 1-3: accumulate on DVE (Pool is busy with the
                    # rms/rope chain tails and would head-of-line block the
                    # next chunk's kT/qT); last block: alternate DVE/Pool
                    on_pool = jm1 == NCHUNK - 1 and (h + tkl) % 2 == 0
                    if tkl == 0:
                        eacc[h] = wpool.tile([128, TQ], F32R, tag=f"eacc{h}",
                                             bufs=1, name=f"eacc{h}_{jm1}")
                        nc.vector.tensor_copy(eacc[h], e_t)
                    elif on_pool:
                        nc.gpsimd.tensor_add(eacc[h][:, c0:], eacc[h][:, c0:],
                                             e_t[:, c0:])
                    else:
                        nc.vector.tensor_add(eacc[h][:, c0:], eacc[h][:, c0:],
                                             e_t[:, c0:])

                def mk_round(tk):
                    def u():
                        tkl = tk - lag
                        if tkl >= 0:
                            for h in pair:
                                consume(h, tkl)
                        if tk < ntk:
                            c0 = tile_c0(tk)
                            for h in pair:
                                s_t = claim_s(f"s{h}_{tk}_{jm1}")
                                nc.tensor.matmul(
                                    s_t[:, c0:],
                                    kTc[(h, tk // 4)][
                                        :, 128 * (tk % 4):128 * (tk % 4) + 128],
                                    qT_saved[(jm1, h)][:, c0:],
                                    start=True,
                                    stop=True,
                                )
                                st[("s", h, tk)] = s_t
                        if tkl >= 0:
                            c0 = tile_c0(tkl)
                            for h in pair:
                                if tkl == 0:
                                    st[("pv", h)] = pspool.tile(
                                        [128, TQ], F32, tag="pv", bufs=2,
                                        name=f"pv{h}_{jm1}")
                                nc.tensor.matmul(
                                    st[("pv", h)][:, c0:],
                                    vst[tkl][:, 128 * h:128 * h + 128],
                                    st[("e", h, tkl)][:, c0:],
                                    start=(tkl == 0),
                                    stop=(tkl == ntk - 1),
                                    skip_group_check=(c0 > 0),
                                )
                    return u

                units += [mk_round(tk) for tk in range(ntk + lag)]

                def tail():
                    for h in pair:
                        se_ps = claim_s(f"se{h}_{jm1}")
                        nc.tensor.matmul(se_ps, ones32, eacc[h],
                                         start=True, stop=True)
                        lnse = wpool.tile([128, TQ], F32, tag="lnse",
                                          name=f"lnse{h}_{jm1}")
                        nc.scalar.activation(lnse, se_ps, AF.Ln)
                        rec = wpool.tile([128, TQ], BF16, tag="rec",
                                         name=f"rec{h}_{jm1}")
                        nc.scalar.activation(rec, lnse, AF.Exp, scale=-1.0)
                        at = apool.tile([128, TQ], BF16, tag=f"at{h}",
                                        bufs=2, name=f"at{h}_{jm1}")
                        nc.vector.tensor_mul(at, st[("pv", h)], rec)
                        at_saved[(jm1, h)] = at

                units.append(tail)
                return units

            def cproj_units(jm1):
                tq0 = TQ * jm1
                units = []
                for co in range(16):
                    def u(co=co):
                        o_ps = pspool.tile([128, TQ], F32, tag="mm", bufs=2,
                                           name=f"ops{co}_{jm1}")
                        for m in range(4):
                            nc.tensor.matmul(
                                o_ps,
                                wp_sb[:, m, 128 * co:128 * co + 128],
                                at_saved[(jm1, m)],
                                start=(m == 0),
                                stop=(m == 3),
                            )
                        o_sb = opool.tile([128, TQ], F16, tag="osb",
                                          name=f"osb{co}_{jm1}")
                        if co % 2 == 0:
                            nc.vector.tensor_copy(o_sb, o_ps)
                        else:
                            nc.scalar.copy(o_sb, o_ps)
                        nc.sync.dma_start(
                            outT[128 * co:128 * co + 128, tq0:tq0 + TQ], o_sb)
                    units.append(u)
                return units

            def run_units(a_units, b_units):
                # units are (closure, weight); merge by cumulative weight
                if not a_units or not b_units:
                    for u, _ in a_units + b_units:
                        u()
                    return
                wa = sum(w for _, w in a_units)
                wb = sum(w for _, w in b_units)
                seq = []
                acc = 0.0
                for i, (u, w) in enumerate(a_units):
                    acc += w
                    seq.append((acc / wa, 0, i, u))
                acc = 0.0
                for i, (u, w) in enumerate(b_units):
                    acc += w
                    seq.append(((acc - 0.5 * w) / wb, 1, i, u))
                seq.sort(key=lambda t: (t[0], t[1], t[2]))
                for _, _, _, u in seq:
                    u()

            def weighted(units, w):
                return [(u, w) for u in units]

            # ---------------- blocks ---------------------------------------
            for j in range(NCHUNK + 1):
                b_units = []
                if j < NCHUNK:
                    if j + 1 < NCHUNK:
                        b_units += weighted(prefetch_unit(j + 1), 0.3)
                    # software-pipeline the rms/rope chains one group back so
                    # the PE never head-of-line blocks on a chain's inputs
                    groups = [qk_group_units(j, "q", m) for m in range(4)]
                    groups += [qk_group_units(j, "k", m) for m in range(4)]
                    groups += [v_group_units(j, tt) for tt in range(4)]
                    pending = None
                    for g in groups:
                        b_units += weighted(g[:-1], 1.0)
                        if pending is not None:
                            b_units += weighted([pending], 1.6)
                        pending = g[-1]
                    b_units += weighted([pending], 1.6)
                else:
                    # last block: chunk-2 c_proj provides the PE filler
                    b_units += weighted(cproj_units(2), 1.0)
                a_units = []
                if j >= 1:
                    lag = 1 if j < NCHUNK else 2
                    if j == NCHUNK:
                        s_rot_state["rot"] = ("s", "ss", "s", "swp")
                    for pair in ((0, 1), (2, 3)):
                        pu = attn_pair_units(j - 1, pair, lag)
                        a_units += weighted(pu[:-1], 1.5)
                        a_units += weighted([pu[-1]], 2.5)
                    if j in (1, 2):
                        a_units += weighted(cproj_units(j - 1), 1.0)
                    elif j == NCHUNK:
                        a_units += weighted(cproj_units(3), 1.0)
                run_units(a_units, b_units)

    nc.finalize()
    return nc


def _host_prep(inputs):
    """Build the 8 per-core input maps (all numpy)."""
    x = np.asarray(inputs["x"], np.float32)
    v1 = np.asarray(inputs["v1"], np.float32)
    x_q = np.asarray(inputs["x_q"], np.float32)
    x_k = np.asarray(inputs["x_k"], np.float32)
    x_v = np.asarray(inputs["x_v"], np.float32)
    Wq = np.asarray(inputs["Wq"], np.float32)
    Wk = np.asarray(inputs["Wk"], np.float32)
    Wv = np.asarray(inputs["Wv"], np.float32)
    Wproj = np.asarray(inputs["Wproj"], np.float32)
    lamb = float(np.asarray(inputs["lamb"]))

    assert np.array_equal(x_q, x_k) and np.array_equal(x_q, x_v), (
        "kernel assumes shared token-shift mix vectors (x_q == x_k == x_v)"
    )

    # token-shift blend, then transpose per batch; pre-tile partition-major:
    # xbig[p, j, kt, t] = xb[b].T[128*kt + p, 512*j + t]
    sh = np.concatenate([np.zeros((B, 1, C), np.float32), x[:, :-1]], axis=1)
    xb = x * (1.0 - x_q) + sh * x_q
    xbig = []
    for b_ in range(B):
        xt = xb[b_].T.astype(_bf)                      # [C, T]
        xt = xt.reshape(NKT, 128, NCHUNK, TQ)          # [kt, p, j, t]
        xbig.append(np.ascontiguousarray(xt.transpose(1, 2, 0, 3)))

    def tile_w(w):  # [C, LC] -> [128, NKT, LC]
        return np.ascontiguousarray(
            w.reshape(NKT, 128, LC).transpose(1, 0, 2))

    # rope tables, duplicated halves; sin second half negated
    inv = 1.0 / (ROPE_THETA ** (np.arange(0, D, 2, dtype=np.float32) / D))
    fr = np.outer(np.arange(T, dtype=np.float32), inv)  # [T, 64]
    cosT = np.cos(fr).T.astype(np.float32)  # [64, T]
    sinT = np.sin(fr).T.astype(np.float32)
    cos2 = np.concatenate([cosT, cosT], axis=0).astype(_bf)
    sin2 = np.concatenate([sinT, -sinT], axis=0).astype(_bf)

    # causal mask master strip: M[p, g] = 0 if g >= p + 384 else MASK_NEG
    p = np.arange(128)[:, None]
    g = np.arange(896)[None, :]
    masks = np.where(g >= p + 384, 0.0, MASK_NEG).astype(np.float32)
    permm = np.roll(np.eye(128, dtype=np.float32), 64, axis=0).astype(_bf)

    in_maps = []
    for c in range(NCORES):
        b_ = c // 4
        g_ = c % 4
        L = slice(LC * g_, LC * g_ + LC)
        # v1big[p, j, tt, c] = (lamb*v1)[512*j + 128*tt + p, L]
        v1l = (lamb * v1[b_][:, L]).astype(_bf)        # [T, LC]
        v1b = np.ascontiguousarray(
            v1l.reshape(NCHUNK, 4, 128, LC).transpose(2, 0, 1, 3))
        # wp[p, m, c] = Wproj[:, L].T[128*m + p, :]
        wpl = np.ascontiguousarray(Wproj[:, L].T)      # [LC, C]
        wpb = np.ascontiguousarray(
            wpl.reshape(4, 128, C).transpose(1, 0, 2)).astype(_bf)
        wq_t = tile_w(np.ascontiguousarray(Wq[L].T).astype(_bf))
        wq_m = np.ascontiguousarray(
            wq_t.reshape(128, NKT, 4, 128).transpose(2, 0, 1, 3))
        in_maps.append({
            "xbig": xbig[b_],
            "wq": wq_m,
            "wk": tile_w(np.ascontiguousarray(Wk[L].T).astype(_bf)),
            "wv": tile_w(
                np.ascontiguousarray(((1.0 - lamb) * Wv[L]).T).astype(_bf)),
            "wp": wpb,
            "v1big": v1b,
            "cos2": cos2,
            "sin2": sin2,
            "masks": masks,
            "perm": permm,
            "ones32": np.ones((128, 128), np.float32),
        })
    return in_maps


def _run(in_maps, trace=False):
    from concourse.bass_utils import run_bass_kernel_spmd

    if "nc" not in _prog_cache:
        _prog_cache["nc"] = _build_program()
    return run_bass_kernel_spmd(
        _prog_cache["nc"], in_maps, core_ids=list(range(NCORES)), trace=trace
    )


def kernel(**inputs) -> np.ndarray:
    residual = np.asarray(inputs["residual"], np.float32)
    in_maps = _host_prep(inputs)
    res = _run(in_maps)
    out = np.empty((B, T, C), np.float32)
    for b_ in range(B):
        acc = res.results[4 * b_]["outT"].astype(np.float32)
        for g_ in range(1, 4):
            acc = acc + res.results[4 * b_ + g_]["outT"]
        out[b_] = residual[b_] + acc.T
    return out
